# revision 2
# baseline (speedup 1.0000x reference)
"""Trainium2 Bass kernel v3: single-head causal attention, one batch el per core.

Structure (per core, per rep):
  - qk projection: w-stationary f32r matmuls -> psum [q;k] -> one DVE copy to
    qk_sb; q/k duplicated onto the opposite partition halves via SBUF DMA.
  - v projection: column-tiled bf16 matmuls computing two T-halves
    concurrently (parts 0-63 / 64-127), copied (cast) to vT bf16, then one
    SBUF->SBUF transpose-DMA gives natural v_sb [k-part, 16, 64] bf16 with a
    preset ones column for the softmax denominator.
  - scores: f32r row-paired (tile_position rows 0/64) k-stationary matmuls in
    sT layout [k, q]; diagonal tiles stream only cols >= d (triangle-exact).
  - exp on ACT reads psum, writes bf16 ets; diagonal members exp only
    [d:512]; a 128-wide staircase multiply on gpsimd applies the causal edge.
  - att@v: v_aug [v|1] bf16 stationary, M=65, accumulating po [65, 512];
    diagonal members stream cols >= d only. Row 64 = softmax denominator.
  - out: po -> oT bf16 [65, 2048]; one transpose-DMA -> nat [128, 16, 65];
    one reciprocal over the 16 denominator columns + 16 scalar muls; DMA out.
"""

import numpy as np

P = 128
B = 8
T = 2048
C = 1024
H = 64
QB = 512
NB = T // QB      # 4
CC = C // P       # 8
KT = T // P       # 16
N_CORES = 8

_CACHE = {}
USE_VT_DMA = False
USE_OUT_DMA = False


def _build(reps=1):
    import concourse.bacc as bacc
    import concourse.mybir as mybir
    import concourse.tile as tile
    from concourse.masks import make_identity

    dt = mybir.dt
    f32 = dt.float32
    f32r = dt.float32r
    bf16 = dt.bfloat16
    AF = mybir.ActivationFunctionType
    ALU = mybir.AluOpType

    nc = bacc.Bacc(None, target_bir_lowering=False)
    xT_d = nc.dram_tensor("xT", [C, T], f32r, kind="ExternalInput")
    xTf_d = nc.dram_tensor("xTf", [C, T], f32, kind="ExternalInput")
    wqk_d = nc.dram_tensor("wqk", [C, 2 * H], f32r, kind="ExternalInput")
    wv_d = nc.dram_tensor("wv", [C, H], f32, kind="ExternalInput")
    out_d = nc.dram_tensor("out", [T, H], f32, kind="ExternalOutput")

    with tile.TileContext(nc) as tc:
        with (
            tc.tile_pool(name="consts", bufs=1) as consts,
            tc.tile_pool(name="xpool", bufs=1) as xpool,
            tc.tile_pool(name="qkvp", bufs=1) as qkvp,
            tc.tile_pool(name="expp", bufs=8) as expp,
            tc.tile_pool(name="otp", bufs=2) as otp,
            tc.tile_pool(name="natp", bufs=2) as natp,
            tc.tile_pool(name="outp", bufs=2) as outp,
            tc.tile_pool(name="psA", bufs=2, space="PSUM") as psA,
            tc.tile_pool(name="psS", bufs=2, space="PSUM") as psS,
            tc.tile_pool(name="psO", bufs=2, space="PSUM") as psO,
        ):
            ident = consts.tile([P, P], f32)
            make_identity(nc, ident)
            ident_bf = consts.tile([P, P], bf16)
            nc.vector.tensor_copy(ident_bf, ident)
            # 128-wide staircase: stair[p, i] = 1.0 if i >= p else 0.0
            stair_f = consts.tile([P, P], f32)
            nc.gpsimd.memset(stair_f, 1.0)
            nc.gpsimd.affine_select(
                out=stair_f, in_=stair_f, compare_op=ALU.is_ge, fill=0.0,
                base=0, pattern=[[1, P]], channel_multiplier=-1,
            )
            stair_bf = consts.tile([P, P], bf16)
            nc.vector.tensor_copy(stair_bf, stair_f)

            wqk_sb = consts.tile([P, CC, 2 * H], f32r)
            nc.sync.dma_start(wqk_sb[:], wqk_d[:, :].rearrange("(c p) h -> p c h", p=P))
            wv_bf = consts.tile([P, CC, H], bf16)
            nc.gpsimd.dma_start(wv_bf[:], wv_d[:, :].rearrange("(c p) h -> p c h", p=P))

            x_sb = xpool.tile([P, CC, T], f32r)
            for bb in range(NB // 2):
                for c in range(CC):
                    nc.sync.dma_start(
                        x_sb[:, c, bb * 2 * QB:(bb + 1) * 2 * QB],
                        xT_d[c * P:(c + 1) * P, bb * 2 * QB:(bb + 1) * 2 * QB],
                    )
            x_bf = xpool.tile([P, CC, T], bf16)
            for c in range(CC):
                nc.gpsimd.dma_start(x_bf[:, c, :], xTf_d[c * P:(c + 1) * P, :])

            # qk_sb: parts 0-63 = q, 64-127 = k; kq_sb: 0-63 = k, 64-127 = q
            qk_sb = qkvp.tile([P, T], f32r)
            kq_sb = qkvp.tile([P, T], f32r)
            vT_bf = qkvp.tile([H, T], bf16)
            v_sb = qkvp.tile([P, KT, H + 1], bf16)
            nc.gpsimd.memset(v_sb[:, :, H:H + 1], 1.0)

            def project_qk(b):
                bsl = slice(b * QB, (b + 1) * QB)
                ps = psA.tile([P, QB], f32, tag="a", name="ps_qk")
                for c in range(CC):
                    nc.tensor.matmul(
                        ps, wqk_sb[:, c, :], x_sb[:, c, bsl],
                        start=(c == 0), stop=(c == CC - 1),
                    )
                nc.vector.tensor_copy(qk_sb[:, bsl], ps)
                nc.sync.dma_start(kq_sb[H:P, bsl], qk_sb[0:H, bsl])
                nc.sync.dma_start(kq_sb[0:H, bsl], qk_sb[H:P, bsl])

            def project_v2(sb2):
                # two T-halves concurrently via column tiling
                tA = slice(sb2 * 2 * QB, sb2 * 2 * QB + QB)
                tB = slice(sb2 * 2 * QB + QB, (sb2 + 1) * 2 * QB)
                pv = psA.tile([P, QB], f32, tag="a", name="ps_v")
                for c in range(CC):
                    nc.tensor.matmul(
                        pv[0:H, :], wv_bf[:, c, :], x_bf[:, c, tA],
                        start=(c == 0), stop=(c == CC - 1),
                        tile_position=(0, 0),
                    )
                    nc.tensor.matmul(
                        pv[H:P, :], wv_bf[:, c, :], x_bf[:, c, tB],
                        start=(c == 0), stop=(c == CC - 1),
                        tile_position=(0, 64),
                    )
                nc.vector.tensor_copy(vT_bf[:, tA], pv[0:H, :])
                nc.vector.tensor_copy(vT_bf[:, tB], pv[H:P, :])

            def v_transpose():
                if USE_VT_DMA:
                    v_nat = natp.tile([P, KT, H], bf16, name="v_nat")
                    nc.sync.dma_start_transpose(v_nat[:, :, :], vT_bf[:, :])
                    nc.vector.tensor_copy(v_sb[:, :, 0:H], v_nat)
                else:
                    for t in range(KT):
                        pv = psA.tile([P, H], bf16, tag="a", name="ps_vt")
                        nc.tensor.matmul(
                            pv, vT_bf[:, t * P:(t + 1) * P], ident_bf[:H, :H],
                            is_transpose=True,
                        )
                        nc.vector.tensor_copy(v_sb[:, t, 0:H], pv)

            def finish(oT, nat, natd):
                rc = natp.tile([P, KT, 1], f32, name="rc")
                nc.vector.reciprocal(rc, natd[:, :, 0:1])
                out_sb = outp.tile([P, KT, H], f32, name="out_sb")
                for t in range(KT):
                    nc.vector.tensor_scalar_mul(
                        out_sb[:, t, :], nat[:, t, :], rc[:, t, :],
                    )
                nc.sync.dma_start(
                    out_d[:, :].rearrange("(g p) h -> p g h", p=P),
                    out_sb[:, :, :],
                )

            def attention(b, oT):
                bsl = slice(b * QB, (b + 1) * QB)
                nk = (b + 1) * 4

                def dof(kc):
                    d = kc * P - b * QB
                    return d if d > 0 else 0

                ets = []
                for j in range(nk // 2):
                    ps2 = psS.tile([P, 2, QB], f32, tag="s", name="ps_s")
                    kc0, kc1 = 2 * j, 2 * j + 1
                    d0, d1 = dof(kc0), dof(kc1)
                    nc.tensor.matmul(
                        ps2[:, 0, d0:], kq_sb[0:H, kc0 * P:(kc0 + 1) * P],
                        qk_sb[0:H, b * QB + d0:(b + 1) * QB], tile_position=(0, 0),
                    )
                    nc.tensor.matmul(
                        ps2[:, 1, d1:], qk_sb[H:P, kc1 * P:(kc1 + 1) * P],
                        kq_sb[H:P, b * QB + d1:(b + 1) * QB], tile_position=(H, 0),
                    )
                    et2 = expp.tile([P, 2, QB], bf16, tag="e", name="et")
                    if d0 == d1:
                        nc.scalar.activation(et2[:, :, d0:], ps2[:, :, d0:], AF.Exp)
                    else:
                        nc.scalar.activation(et2[:, 0, d0:], ps2[:, 0, d0:], AF.Exp)
                        nc.scalar.activation(et2[:, 1, d1:], ps2[:, 1, d1:], AF.Exp)
                    for jj in range(2):
                        kc = 2 * j + jj
                        d = kc * P - b * QB
                        if d >= 0:  # diagonal tile -> causal staircase
                            nc.gpsimd.tensor_mul(
                                et2[:, jj, d:d + P], et2[:, jj, d:d + P], stair_bf,
                            )
                    ets.append(et2)

                def et(kc):
                    return ets[kc // 2][:, kc % 2, :]

                po = psO.tile([H + 1, QB], f32, tag="o", name="ps_av")
                for kc in range(nk):
                    d = dof(kc)
                    nc.tensor.matmul(po[:, d:], v_sb[:, kc, :], et(kc)[:, d:],
                                     start=(kc == 0), stop=(kc == nk - 1))
                nc.vector.tensor_copy(oT[0:H + 1, bsl], po)

            for _rep in range(reps):
                for b in range(NB):
                    project_qk(b)
                for sb2 in range(NB // 2):
                    project_v2(sb2)
                v_transpose()
                oT = otp.tile([H + 32, T], bf16, name="oT")
                nat = natp.tile([P, KT, H], bf16, name="nat")
                natd = natp.tile([P, KT, 32], bf16, name="natd")
                if USE_OUT_DMA:
                    for b in range(NB):
                        attention(b, oT)
                        bsl = slice(b * QB, (b + 1) * QB)
                        nc.sync.dma_start_transpose(
                            nat[:, b * 4:(b + 1) * 4, :], oT[0:H, bsl])
                        nc.sync.dma_start_transpose(
                            natd[:, b * 4:(b + 1) * 4, :], oT[H:H + 32, bsl])
                    finish(oT, nat, natd)
                else:
                    out_sb = outp.tile([P, KT, H], f32, name="out_sb")
                    for b in range(NB):
                        attention(b, oT)
                        for st in range(4):
                            t = b * 4 + st
                            pt = psO.tile([P, H + 1], bf16, tag="o", name="ps_t")
                            nc.tensor.matmul(
                                pt, oT[0:H + 1, t * P:(t + 1) * P],
                                ident_bf[:H + 1, :H + 1], is_transpose=True,
                            )
                            rc = natp.tile([P, 1], f32, name="rcp")
                            nc.vector.reciprocal(rc, pt[:, H:H + 1])
                            nc.vector.tensor_scalar_mul(out_sb[:, t, :], pt[:, 0:H], rc)
                        nc.sync.dma_start(
                            out_d[:, :].rearrange("(g p) h -> p g h", p=P)[:, b * 4:(b + 1) * 4, :],
                            out_sb[:, b * 4:(b + 1) * 4, :],
                        )

    nc.compile()
    return nc


def _get_nc():
    nc = _CACHE.get("nc")
    if nc is None:
        nc = _build()
        _CACHE["nc"] = nc
    return nc


def _make_in_maps(inputs):
    x = np.asarray(inputs["x"], dtype=np.float32)
    Wq = np.asarray(inputs["Wq"], dtype=np.float32)
    Wk = np.asarray(inputs["Wk"], dtype=np.float32)
    Wv = np.asarray(inputs["Wv"], dtype=np.float32)
    scale = np.float32(1.0 / np.sqrt(np.float32(Wq.shape[1])))
    wqk = np.ascontiguousarray(
        np.concatenate([Wq * scale, Wk], axis=1), dtype=np.float32)
    wv_c = np.ascontiguousarray(Wv, dtype=np.float32)
    in_maps = []
    for b in range(N_CORES):
        xt = np.ascontiguousarray(x[b].T)
        in_maps.append({
            "xT": xt,
            "xTf": xt,
            "wqk": wqk,
            "wv": wv_c,
        })
    return in_maps


def _run(inputs, **kwargs):
    from concourse.bass_utils import run_bass_kernel_spmd

    nc = _get_nc()
    res = run_bass_kernel_spmd(nc, _make_in_maps(inputs), core_ids=list(range(N_CORES)), **kwargs)
    out = np.stack([res.results[i]["out"] for i in range(N_CORES)], axis=0)
    return out.astype(np.float32, copy=False), res


def kernel(**inputs):
    out, _ = _run(inputs)
    return out


def kernel_profiled(**inputs):
    """Returns (out, BassKernelResults)."""
    out, res = _run(inputs)
    return out, res
